# revision 5
# baseline (speedup 1.0000x reference)
"""CASSI forward A^T(A(x)) kernel for Trainium2, 8-core data parallel.

Reference computation (independent per batch b and row m):
    y1[l, n]  = x[b, l, m, n] * phi[l, m, n]
    y2[j]     = sum_l y1[l, j - 2l]              (j in [0, 310))
    out[l, n] = phi[l, m, n] * y2[2l + n]

Precision: the harness gate is rel_err < 2e-2 against the f32 reference;
bf16 keeps worst-case element error ~0.5%, so the whole pipeline runs in
bf16 — including HBM.  The host casts x/phi to bf16 and pre-transposes to
m-major [.., M, L*N] so every DMA is a fully-contiguous 14336B-per-
partition transfer; the kernel stores bf16 m-major output and the host
transposes/casts back.  This halves HBM traffic (66 -> 33 MB per core)
and doubles DVE throughput (2x perf mode).

On-chip layout: partitions = rows m (two 128-row tiles per batch), free
dim = (l, n).  The 28-band shift-scatter-add runs as a 5-level binary tree
of strided DVE adds.  The mask-mul writes y1 into a scratch tile laid out
with small zero gaps between paired bands so each tree level is a single
(or two) wide strided tensor_tensor op whose shifted operand reads zeros
where a block has no data — no aliased read-modify-write, no per-band op
chain.  Gaps are memset once at kernel start; level ops rewrite only data
regions.

Uniform-slot layout: at every level, slot width = data width + next-level
shift, so in0's right-pad zeros and in1's left-pad zeros are the SAME gap
cells and every level op is a plain 2-free-dim strided tensor_tensor:
  y1  band l (256) at 258*l                        gaps [256,258) per slot
  u   i=0..13 (258) at 262*i                       gaps [258,262)
  q   i=0..6  (262) at 278*i                       gaps [262,278), [1930,1938)
  o   i=0..2  (270) at 286*i                       gaps [270,286), [842,850)
  s   s0 (286) at 0, m1 (278) at 342               zeros [286,342)
  y2  (310) dense
The final re-mask writes a dense [P, L*N] tile (recycling the x tile) so
stores are fully contiguous on both SBUF and DRAM sides.

Sharding: batch dim (32) split 4-per-core across 8 cores; phi replicated.
"""

import numpy as np

B, L, M, N = 32, 28, 256, 256
STRIDE = 2
NCORES = 8
BPC = B // NCORES            # batches per core
NOUT = N + STRIDE * (L - 1)  # 310
P = 128                      # partitions per row tile
LN = L * N                   # 7168
Y1_W = 258 * 28              # 7224, band l at 258*l, gaps [256,258) per slot
U_W = 262 * 14               # 3668, u_i at 262*i, gaps [258,262)
Q_W = 1938                   # q_i at 278*i (uniform); gaps [262,278), [1930,1938)
O_W = 850                    # o_i at 286*i; zeros [270,286)x2, [842,850)
S_W = 620                    # s0@0 (286), zeros [286,342), m1@342 (278)
XT_BUFS = 3
G_MM = 8      # bands of the mask-mul offloaded to gpsimd (DVE does the rest)
G_FM = 8      # bands of the final re-mask offloaded to gpsimd

_cached = {}


def _build_nc():
    import concourse.bass as bass
    import concourse.mybir as mybir
    from concourse.ap import AP
    from concourse.tile import TileContext

    bf16 = mybir.dt.bfloat16
    nc = bass.Bass()
    x = nc.dram_tensor("x", [BPC, M, LN], bf16, kind="ExternalInput")
    phi = nc.dram_tensor("phi", [M, LN], bf16, kind="ExternalInput")
    out = nc.dram_tensor("out", [BPC, M, LN], bf16, kind="ExternalOutput")

    def sub(t, off, dims):
        """AP over tile t at element offset off with free dims [[step,count],..]."""
        full = t[:]
        return AP(full.tensor, full.offset + off,
                  [[full.ap[0][0], P]] + [list(d) for d in dims])

    with TileContext(nc) as tc:
        with (
            tc.tile_pool(name="phipool", bufs=1) as phipool,
            tc.tile_pool(name="xpool", bufs=1) as xpool,
            tc.tile_pool(name="scratch", bufs=1) as sp,
        ):
            # --- persistent tiles ------------------------------------------------
            phit = [phipool.tile([P, LN], bf16, name=f"phi{pt}", tag=f"phi{pt}")
                    for pt in range(M // P)]
            xts = [xpool.tile([P, LN], bf16, name=f"xt{i}", tag=f"xt{i}")
                   for i in range(XT_BUFS)]
            y1t = sp.tile([P, Y1_W], bf16, name="y1", tag="y1")
            ut = sp.tile([P, U_W], bf16, name="u", tag="u")
            qt = sp.tile([P, Q_W], bf16, name="q", tag="q")
            ot = sp.tile([P, O_W], bf16, name="o", tag="o")
            st = sp.tile([P, S_W], bf16, name="s", tag="s")
            y2t = sp.tile([P, NOUT], bf16, name="y2", tag="y2")

            # --- one-time zero-gap memsets (never written afterwards) ------------
            nc.vector.memset(sub(y1t, 256, [[258, 28], [1, 2]]), 0.0)
            nc.vector.memset(sub(ut, 258, [[262, 14], [1, 4]]), 0.0)
            nc.vector.memset(sub(qt, 262, [[278, 6], [1, 16]]), 0.0)
            nc.vector.memset(sub(qt, 1930, [[1, 8]]), 0.0)
            nc.vector.memset(sub(ot, 270, [[286, 2], [1, 16]]), 0.0)
            nc.vector.memset(sub(ot, 842, [[1, 8]]), 0.0)
            nc.vector.memset(sub(st, 286, [[1, 56]]), 0.0)

            # --- first-tile loads quarter-split by bands so the first mask-mul
            # can start after ~0.9 MB instead of 3.67 MB ------------------------
            CH = 7 * N  # 7-band chunk
            def dram_chunk(rows_ap, c):
                return AP(rows_ap.tensor, rows_ap.offset + c * CH,
                          [list(rows_ap.ap[0]), [1, CH]])
            for c in range(4):
                nc.sync.dma_start(out=sub(phit[0], c * CH, [[1, CH]]),
                                  in_=dram_chunk(phi[0:P], c))
                nc.scalar.dma_start(out=sub(xts[0], c * CH, [[1, CH]]),
                                    in_=dram_chunk(x[0][0:P], c))

            it = 0
            for pt in range(M // P):
                for b in range(BPC):
                    xt = xts[it % XT_BUFS]
                    it += 1
                    if it == 2:
                        # phi1 on the SP ring: balances rings at 15.6 MB each
                        # (ACT carries all 8 x loads)
                        nc.sync.dma_start(out=phit[1][:], in_=phi[P: 2 * P])
                    if it > 1:
                        nc.scalar.dma_start(
                            out=xt[:], in_=x[b][pt * P: (pt + 1) * P])
                    # y1 = x * phi, dense -> uniform gapped scratch; a slice
                    # of bands runs on gpsimd in parallel with the DVE slice
                    D_MM = L - G_MM
                    if it == 1:
                        # quarter-split to chase the chunked loads
                        for c in range(4):
                            nc.vector.tensor_mul(
                                out=sub(y1t, 258 * 7 * c, [[258, 7], [1, 256]]),
                                in0=sub(xt, c * CH, [[256, 7], [1, 256]]),
                                in1=sub(phit[pt], c * CH, [[256, 7], [1, 256]]),
                            )
                    else:
                        nc.vector.tensor_mul(
                            out=sub(y1t, 0, [[258, D_MM], [1, 256]]),
                            in0=sub(xt, 0, [[256, D_MM], [1, 256]]),
                            in1=sub(phit[pt], 0, [[256, D_MM], [1, 256]]),
                        )
                        nc.gpsimd.tensor_mul(
                            out=sub(y1t, 258 * D_MM, [[258, G_MM], [1, 256]]),
                            in0=sub(xt, 256 * D_MM, [[256, G_MM], [1, 256]]),
                            in1=sub(phit[pt], 256 * D_MM, [[256, G_MM], [1, 256]]),
                        )
                    # L1: 14 pair-sums -> u
                    nc.vector.tensor_add(
                        out=sub(ut, 0, [[262, 14], [1, 258]]),
                        in0=sub(y1t, 0, [[516, 14], [1, 258]]),
                        in1=sub(y1t, 256, [[516, 14], [1, 258]]),
                    )
                    # L2: 7 quad-sums -> q (single uniform op, stride 278)
                    nc.vector.tensor_add(
                        out=sub(qt, 0, [[278, 7], [1, 262]]),
                        in0=sub(ut, 0, [[524, 7], [1, 262]]),
                        in1=sub(ut, 258, [[524, 7], [1, 262]]),
                    )
                    # L3: 3 oct-sums -> o
                    nc.vector.tensor_add(
                        out=sub(ot, 0, [[286, 3], [1, 270]]),
                        in0=sub(qt, 0, [[556, 3], [1, 270]]),
                        in1=sub(qt, 270, [[556, 3], [1, 270]]),
                    )
                    # L4: s0 = o0 + shift16(o1); m1 = o2 + shift16(q6)
                    nc.vector.tensor_add(
                        out=sub(st, 0, [[1, 286]]),
                        in0=sub(ot, 0, [[1, 286]]),
                        in1=sub(ot, 270, [[1, 286]]),
                    )
                    nc.vector.tensor_add(
                        out=sub(st, 342, [[1, 278]]),
                        in0=sub(ot, 572, [[1, 278]]),
                        in1=sub(qt, 1652, [[1, 278]]),
                    )
                    # L5: y2 = s0 + shift32(m1)
                    nc.vector.tensor_add(
                        out=sub(y2t, 0, [[1, 310]]),
                        in0=sub(st, 0, [[1, 310]]),
                        in1=sub(st, 310, [[1, 310]]),
                    )
                    # out = phi * gather(y2), written dense into the consumed
                    # x tile so the store is contiguous on both sides; a band
                    # slice runs on gpsimd
                    D_FM = L - G_FM
                    nc.vector.tensor_mul(
                        out=sub(xt, 0, [[256, D_FM], [1, 256]]),
                        in0=sub(y2t, 0, [[2, D_FM], [1, 256]]),
                        in1=sub(phit[pt], 0, [[256, D_FM], [1, 256]]),
                    )
                    nc.gpsimd.tensor_mul(
                        out=sub(xt, 256 * D_FM, [[256, G_FM], [1, 256]]),
                        in0=sub(y2t, 2 * D_FM, [[2, G_FM], [1, 256]]),
                        in1=sub(phit[pt], 256 * D_FM, [[256, G_FM], [1, 256]]),
                    )
                    o_rows = out[b][pt * P: (pt + 1) * P]
                    if it <= 2 * BPC - 3:
                        # full store on the SP ring (balances the ACT ring's
                        # loads: ~15.6 MB per ring)
                        nc.sync.dma_start(out=o_rows, in_=xt[:])
                    else:
                        # last stores split across both rings to shrink the
                        # tail drain (ACT's loads are done by then)
                        half = LN // 2
                        for par, eng in ((0, nc.sync), (1, nc.scalar)):
                            eng.dma_start(
                                out=AP(o_rows.tensor, o_rows.offset + half * par,
                                       [list(o_rows.ap[0]), [1, half]]),
                                in_=sub(xt, half * par, [[1, half]]),
                            )
    _split_excess_waits(nc, mybir)
    return nc


def _split_excess_waits(nc, mybir):
    """Move all-but-one semaphore waits off capacity-limited instructions.

    The TRN2 ISA packs sync commands into each 64B instruction; multi-dim
    TT/DMA encodings have room for only one wait, and walrus codegen dies
    with "Too many sync wait commands" instead of splitting.  A standalone
    EventSemaphore on the same engine right before the op is semantically
    identical (the sequencer executes both in order)."""
    ctr = 0
    for bb in nc.m.functions[0].blocks:
        new = []
        for ins in bb.instructions:
            si = ins.sync_info
            waits = list(si.on_wait) if si is not None and si.on_wait else []
            if len(waits) > 1:
                for w in waits[:-1]:
                    ctr += 1
                    new.append(mybir.InstEventSemaphore(
                        name=f"wsplit-{ctr}",
                        engine=ins.engine,
                        sync_info=mybir.SyncInfo(on_wait=[w], on_update=[]),
                    ))
                ins.sync_info = mybir.SyncInfo(
                    on_wait=[waits[-1]],
                    on_update=list(si.on_update or []),
                )
            new.append(ins)
        bb.instructions = new


def _get_nc():
    if "nc" not in _cached:
        _cached["nc"] = _build_nc()
    return _cached["nc"]


def make_in_maps(x: np.ndarray, phi: np.ndarray):
    """Host-side prep: cast to bf16, transpose to m-major, shard batches."""
    import ml_dtypes

    bf = ml_dtypes.bfloat16
    # [B, L, M, N] -> [B, M, L*N], bf16
    xb = np.ascontiguousarray(
        x.astype(bf).transpose(0, 2, 1, 3)).reshape(B, M, LN)
    phib = np.ascontiguousarray(
        phi.astype(bf).transpose(1, 0, 2)).reshape(M, LN)
    return [
        {"x": xb[c * BPC: (c + 1) * BPC], "phi": phib} for c in range(NCORES)
    ]


def postprocess(outs) -> np.ndarray:
    """[BPC, M, L*N] bf16 per core -> full [B, L, M, N] f32."""
    full = np.concatenate(outs, axis=0).reshape(B, M, L, N)
    return np.ascontiguousarray(full.transpose(0, 2, 1, 3)).astype(np.float32)


def kernel(x: np.ndarray, phi: np.ndarray) -> np.ndarray:
    from concourse.bass_utils import run_bass_kernel_spmd

    x = np.ascontiguousarray(x, dtype=np.float32)
    phi = np.ascontiguousarray(phi, dtype=np.float32)
    assert x.shape == (B, L, M, N) and phi.shape == (L, M, N)

    nc = _get_nc()
    in_maps = make_in_maps(x, phi)
    res = run_bass_kernel_spmd(nc, in_maps, core_ids=list(range(NCORES)))
    return postprocess([res.results[c]["out"] for c in range(NCORES)])


# revision 13
# speedup vs baseline: 1.2348x; 1.2348x over previous
"""CASSI forward A^T(A(x)) kernel for Trainium2, 8-core data parallel.

Reference computation (independent per batch b and row m):
    y1[l, n]  = x[b, l, m, n] * phi[l, m, n]
    y2[j]     = sum_l y1[l, j - 2l]              (j in [0, 310))
    out[l, n] = phi[l, m, n] * y2[2l + n]

Precision: the harness gate is rel_err < 2e-2 against the f32 reference;
bf16 keeps worst-case element error ~0.5%, so the whole pipeline runs in
bf16 — including HBM.  The host casts x/phi to bf16 and pre-transposes to
m-major [.., M, L*N] so every DMA is a fully-contiguous 14336B-per-
partition transfer; the kernel stores bf16 m-major output and the host
transposes/casts back.  This halves HBM traffic (66 -> 33 MB per core)
and doubles DVE throughput (2x perf mode).

On-chip layout: partitions = rows m (two 128-row tiles per batch), free
dim = (l, n).  The 28-band shift-scatter-add runs as a 5-level binary tree
of strided DVE adds.  The mask-mul writes y1 into a scratch tile laid out
with small zero gaps between paired bands so each tree level is a single
(or two) wide strided tensor_tensor op whose shifted operand reads zeros
where a block has no data — no aliased read-modify-write, no per-band op
chain.  Gaps are memset once at kernel start; level ops rewrite only data
regions.

Uniform-slot layout: at every level, slot width = data width + next-level
shift, so in0's right-pad zeros and in1's left-pad zeros are the SAME gap
cells and every level op is a plain 2-free-dim strided tensor_tensor:
  y1  band l (256) at 258*l                        gaps [256,258) per slot
  u   i=0..13 (258) at 262*i                       gaps [258,262)
  q   i=0..6  (262) at 278*i                       gaps [262,278), [1930,1938)
  o   i=0..2  (270) at 286*i                       gaps [270,286), [842,850)
  s   s0 (286) at 0, m1 (278) at 342               zeros [286,342)
  y2  (310) dense
The final re-mask writes a dense [P, L*N] tile (recycling the x tile) so
stores are fully contiguous on both SBUF and DRAM sides.

Sharding: batch dim (32) split 4-per-core across 8 cores; phi replicated.
"""

import numpy as np

B, L, M, N = 32, 28, 256, 256
STRIDE = 2
NCORES = 8
BPC = B // NCORES            # batches per core
NOUT = N + STRIDE * (L - 1)  # 310
P = 128                      # partitions per row tile
LN = L * N                   # 7168
Y1_W = 258 * 28              # 7224, band l at 258*l, gaps [256,258) per slot
U_W = 262 * 14               # 3668, u_i at 262*i, gaps [258,262)
Q_W = 1938                   # q_i at 278*i (uniform); gaps [262,278), [1930,1938)
O_W = 850                    # o_i at 286*i; zeros [270,286)x2, [842,850)
S_W = 620                    # s0@0 (286), zeros [286,342), m1@342 (278)
XT_BUFS = 3
# NOTE: offloading mul bands to gpsimd was tried and REGRESSED (130->165us):
# concurrent gpsimd tensor ops contend for the shared SBUF port and slow the
# DVE's 2x bf16 mode down ~1.25x.  Keep all elementwise work on DVE.
G_MM = 0      # bands of the mask-mul offloaded to gpsimd (DVE does the rest)
G_FM = 0      # bands of the final re-mask offloaded to gpsimd

_cached = {}


def _build_nc(bphi: bool = False):
    """bphi=True: phi is band-broadcast (phi[l]==phi[0] for all l, as in the
    reference's setup_inputs).  Load only the 2D mask and read it with
    stride-0 band-broadcast APs — saves 3.5 MB of HBM loads per core."""
    import concourse.bass as bass
    import concourse.mybir as mybir
    from concourse.ap import AP
    from concourse.tile import TileContext

    bf16 = mybir.dt.bfloat16
    nc = bass.Bass()
    x = nc.dram_tensor("x", [BPC, M, LN], bf16, kind="ExternalInput")
    PHI_W = N if bphi else LN
    phi = nc.dram_tensor("phi", [M, PHI_W], bf16, kind="ExternalInput")
    out = nc.dram_tensor("out", [BPC, M, LN], bf16, kind="ExternalOutput")

    def sub(t, off, dims):
        """AP over tile t at element offset off with free dims [[step,count],..]."""
        full = t[:]
        return AP(full.tensor, full.offset + off,
                  [[full.ap[0][0], P]] + [list(d) for d in dims])

    # phi AP helper: band-slice of the phi tile for [bands, 256]-shaped ops.
    # In bphi mode the tile holds one 256-wide mask row, broadcast via step 0.
    PB = 0 if bphi else 256

    def phi_ap(pt_tile, band0, nbands):
        return sub(pt_tile, PB * band0, [[PB, nbands], [1, 256]])

    with TileContext(nc) as tc:
        with (
            tc.tile_pool(name="phipool", bufs=1) as phipool,
            tc.tile_pool(name="xpool", bufs=1) as xpool,
            tc.tile_pool(name="scratch", bufs=1) as sp,
        ):
            # --- persistent tiles ------------------------------------------------
            phit = [phipool.tile([P, PHI_W], bf16, name=f"phi{pt}", tag=f"phi{pt}")
                    for pt in range(M // P)]
            xts = [xpool.tile([P, LN], bf16, name=f"xt{i}", tag=f"xt{i}")
                   for i in range(XT_BUFS)]
            y1t = sp.tile([P, Y1_W], bf16, name="y1", tag="y1")
            ut = sp.tile([P, U_W], bf16, name="u", tag="u")
            qt = sp.tile([P, Q_W], bf16, name="q", tag="q")
            ot = sp.tile([P, O_W], bf16, name="o", tag="o")
            st = sp.tile([P, S_W], bf16, name="s", tag="s")
            y2t = sp.tile([P, NOUT], bf16, name="y2", tag="y2")

            # --- one-time zero-gap memsets (never written afterwards) ------------
            nc.vector.memset(sub(y1t, 256, [[258, 28], [1, 2]]), 0.0)
            nc.vector.memset(sub(ut, 258, [[262, 14], [1, 4]]), 0.0)
            nc.vector.memset(sub(qt, 262, [[278, 6], [1, 16]]), 0.0)
            nc.vector.memset(sub(qt, 1930, [[1, 8]]), 0.0)
            nc.vector.memset(sub(ot, 270, [[286, 2], [1, 16]]), 0.0)
            nc.vector.memset(sub(ot, 842, [[1, 8]]), 0.0)
            nc.vector.memset(sub(st, 286, [[1, 56]]), 0.0)

            # --- first-tile loads quarter-split by bands so the first mask-mul
            # can start after ~0.9 MB instead of 3.67 MB ------------------------
            CH = 7 * N  # 7-band chunk
            def dram_chunk(rows_ap, c):
                return AP(rows_ap.tensor, rows_ap.offset + c * CH,
                          [list(rows_ap.ap[0]), [1, CH]])
            if bphi:
                nc.sync.dma_start(out=phit[0][:], in_=phi[0:P])
            else:
                for c in range(4):
                    nc.sync.dma_start(out=sub(phit[0], c * CH, [[1, CH]]),
                                      in_=dram_chunk(phi[0:P], c))
            for c in range(4):
                nc.scalar.dma_start(out=sub(xts[0], c * CH, [[1, CH]]),
                                    in_=dram_chunk(x[0][0:P], c))

            it = 0
            for pt in range(M // P):
                for b in range(BPC):
                    xt = xts[it % XT_BUFS]
                    it += 1
                    if it == 2:
                        # phi1 on the SP ring: balances rings at 15.6 MB each
                        # (ACT carries all 8 x loads)
                        nc.sync.dma_start(out=phit[1][:], in_=phi[P: 2 * P])
                    if it > 1:
                        nc.scalar.dma_start(
                            out=xt[:], in_=x[b][pt * P: (pt + 1) * P])
                    # y1 = x * phi, dense -> uniform gapped scratch; a slice
                    # of bands runs on gpsimd in parallel with the DVE slice
                    D_MM = L - G_MM
                    if it == 1:
                        # quarter-split to chase the chunked loads
                        for c in range(4):
                            nc.vector.tensor_mul(
                                out=sub(y1t, 258 * 7 * c, [[258, 7], [1, 256]]),
                                in0=sub(xt, c * CH, [[256, 7], [1, 256]]),
                                in1=phi_ap(phit[pt], 7 * c, 7),
                            )
                    else:
                        nc.vector.tensor_mul(
                            out=sub(y1t, 0, [[258, D_MM], [1, 256]]),
                            in0=sub(xt, 0, [[256, D_MM], [1, 256]]),
                            in1=phi_ap(phit[pt], 0, D_MM),
                        )
                        if G_MM:
                            nc.gpsimd.tensor_mul(
                                out=sub(y1t, 258 * D_MM, [[258, G_MM], [1, 256]]),
                                in0=sub(xt, 256 * D_MM, [[256, G_MM], [1, 256]]),
                                in1=phi_ap(phit[pt], D_MM, G_MM),
                            )
                    # L1: 14 pair-sums -> u
                    nc.vector.tensor_add(
                        out=sub(ut, 0, [[262, 14], [1, 258]]),
                        in0=sub(y1t, 0, [[516, 14], [1, 258]]),
                        in1=sub(y1t, 256, [[516, 14], [1, 258]]),
                    )
                    # L2: 7 quad-sums -> q (single uniform op, stride 278)
                    nc.vector.tensor_add(
                        out=sub(qt, 0, [[278, 7], [1, 262]]),
                        in0=sub(ut, 0, [[524, 7], [1, 262]]),
                        in1=sub(ut, 258, [[524, 7], [1, 262]]),
                    )
                    # L3: 3 oct-sums -> o
                    nc.vector.tensor_add(
                        out=sub(ot, 0, [[286, 3], [1, 270]]),
                        in0=sub(qt, 0, [[556, 3], [1, 270]]),
                        in1=sub(qt, 270, [[556, 3], [1, 270]]),
                    )
                    # L4: s0 = o0 + shift16(o1); m1 = o2 + shift16(q6)
                    nc.vector.tensor_add(
                        out=sub(st, 0, [[1, 286]]),
                        in0=sub(ot, 0, [[1, 286]]),
                        in1=sub(ot, 270, [[1, 286]]),
                    )
                    nc.vector.tensor_add(
                        out=sub(st, 342, [[1, 278]]),
                        in0=sub(ot, 572, [[1, 278]]),
                        in1=sub(qt, 1652, [[1, 278]]),
                    )
                    # L5: y2 = s0 + shift32(m1)
                    nc.vector.tensor_add(
                        out=sub(y2t, 0, [[1, 310]]),
                        in0=sub(st, 0, [[1, 310]]),
                        in1=sub(st, 310, [[1, 310]]),
                    )
                    # out = phi * gather(y2), written dense into the consumed
                    # x tile so the store is contiguous on both sides; a band
                    # slice runs on gpsimd
                    D_FM = L - G_FM
                    nc.vector.tensor_mul(
                        out=sub(xt, 0, [[256, D_FM], [1, 256]]),
                        in0=sub(y2t, 0, [[2, D_FM], [1, 256]]),
                        in1=phi_ap(phit[pt], 0, D_FM),
                    )
                    if G_FM:
                        nc.gpsimd.tensor_mul(
                            out=sub(xt, 256 * D_FM, [[256, G_FM], [1, 256]]),
                            in0=sub(y2t, 2 * D_FM, [[2, G_FM], [1, 256]]),
                            in1=phi_ap(phit[pt], D_FM, G_FM),
                        )
                    o_rows = out[b][pt * P: (pt + 1) * P]
                    if it <= 2 * BPC - 3:
                        # full store on the SP ring (balances the ACT ring's
                        # loads: ~15.6 MB per ring)
                        nc.sync.dma_start(out=o_rows, in_=xt[:])
                    else:
                        # last stores split across both rings to shrink the
                        # tail drain (ACT's loads are done by then)
                        half = LN // 2
                        for par, eng in ((0, nc.sync), (1, nc.scalar)):
                            eng.dma_start(
                                out=AP(o_rows.tensor, o_rows.offset + half * par,
                                       [list(o_rows.ap[0]), [1, half]]),
                                in_=sub(xt, half * par, [[1, half]]),
                            )
    _split_excess_waits(nc, mybir)
    return nc


def _split_excess_waits(nc, mybir):
    """Move all-but-one semaphore waits off capacity-limited instructions.

    The TRN2 ISA packs sync commands into each 64B instruction; multi-dim
    TT/DMA encodings have room for only one wait, and walrus codegen dies
    with "Too many sync wait commands" instead of splitting.  A standalone
    EventSemaphore on the same engine right before the op is semantically
    identical (the sequencer executes both in order)."""
    ctr = 0
    for bb in nc.m.functions[0].blocks:
        new = []
        for ins in bb.instructions:
            si = ins.sync_info
            waits = list(si.on_wait) if si is not None and si.on_wait else []
            if len(waits) > 1:
                for w in waits[:-1]:
                    ctr += 1
                    new.append(mybir.InstEventSemaphore(
                        name=f"wsplit-{ctr}",
                        engine=ins.engine,
                        sync_info=mybir.SyncInfo(on_wait=[w], on_update=[]),
                    ))
                ins.sync_info = mybir.SyncInfo(
                    on_wait=[waits[-1]],
                    on_update=list(si.on_update or []),
                )
            new.append(ins)
        bb.instructions = new


def _get_nc(bphi: bool = False):
    key = ("nc", bphi)
    if key not in _cached:
        _cached[key] = _build_nc(bphi)
    return _cached[key]


def is_band_broadcast(phi: np.ndarray) -> bool:
    return bool(np.all(phi[0:1] == phi))


def make_in_maps(x: np.ndarray, phi: np.ndarray, bphi: bool = False):
    """Host-side prep: cast to bf16, transpose to m-major, shard batches."""
    import ml_dtypes

    bf = ml_dtypes.bfloat16
    # [B, L, M, N] -> [B, M, L*N], bf16
    xb = np.ascontiguousarray(
        x.astype(bf).transpose(0, 2, 1, 3)).reshape(B, M, LN)
    if bphi:
        phib = np.ascontiguousarray(phi[0].astype(bf))          # [M, N]
    else:
        phib = np.ascontiguousarray(
            phi.astype(bf).transpose(1, 0, 2)).reshape(M, LN)
    return [
        {"x": xb[c * BPC: (c + 1) * BPC], "phi": phib} for c in range(NCORES)
    ]


def postprocess(outs) -> np.ndarray:
    """[BPC, M, L*N] bf16 per core -> full [B, L, M, N] f32."""
    full = np.concatenate(outs, axis=0).reshape(B, M, L, N)
    return np.ascontiguousarray(full.transpose(0, 2, 1, 3)).astype(np.float32)


def kernel(x: np.ndarray, phi: np.ndarray) -> np.ndarray:
    from concourse.bass_utils import run_bass_kernel_spmd

    x = np.ascontiguousarray(x, dtype=np.float32)
    phi = np.ascontiguousarray(phi, dtype=np.float32)
    assert x.shape == (B, L, M, N) and phi.shape == (L, M, N)

    bphi = is_band_broadcast(phi)
    nc = _get_nc(bphi)
    in_maps = make_in_maps(x, phi, bphi)
    res = run_bass_kernel_spmd(nc, in_maps, core_ids=list(range(NCORES)))
    return postprocess([res.results[c]["out"] for c in range(NCORES)])
